# revision 19
# baseline (speedup 1.0000x reference)
"""Trainium2 Bass kernel for BottleneckedEnsembleAttention.

Sharding: 8 cores, core c handles heads [2c, 2c+1] for both batches
(4 independent (b, head) attention problems per core).

All-bf16 datapath (fp32 PSUM accumulation, fp32 scalars). Host precomputes:
  - X^T per (b,h) in bf16 ([8, 128, T] d-major tiles) -- no on-device
    transposes needed.
  - a pair-interleaved permutation of the head dim (u' = 2j <-> q[j],
    2j+1 <-> q[j+32]) applied to Wq/Wk columns and the YaRN cos/sin tables.
    Scores are invariant under any fixed permutation of the contraction dim,
    and in this layout rotate-half becomes a within-32-partition swap
    (partner = u XOR 1) which the DVE stream_shuffle does in ONE op --
    eliminating the second (rotated-weight) projection matmul pass.
  - sin table carries the rotate-half signs; cos/sin q-rows carry the
    softmax scale.

Per (b, h) on-device pipeline:
  1. load X^T tiles (bf16), per 512-col chunk: psum = [Wq|Wk]^T X^T,
     evict bf16, stream_shuffle, RoPE mul/add (Pool + DVE), partition-swap
     DMA to build kq (k rows 0-63 / q rows 64-127).
  2. v in natural layout: [s-tile, 64] = xt_tile^T @ Wv accumulated over
     d-chunks directly in PSUM col-slices (8 s-tiles per bank), plus a
     ones-column for the softmax denominator.
  3. per t-chunk (512 cols): scores^T = k lhsT @ q rhs with causal tail
     matmuls on diagonal blocks (cols [kd*128:512] only), exp via ACT
     (bias = -1e30 for inactive s), triangle mask on the diagonal 128-block
     (Pool), att^T accumulated over s-tiles with matching tail widths,
     o_proj per t-tile, eviction scaled by active[t]/denom[t] (DVE/ACT),
     batched 4-tile output store (bf16).

Emission is software-pipelined: pair i+1's projection/RoPE/v steps are
interleaved into pair i's attention chunks as PE gap filler; each chunk's
o_proj finisher is deferred one chunk.
"""

import math
from contextlib import ExitStack

import numpy as np
import ml_dtypes

import concourse.bass as bass
import concourse.mybir as mybir
import concourse.tile as tile
from concourse import bacc
from concourse.bass_utils import run_bass_kernel_spmd

# model constants (must match reference.py)
HIDDEN = 1024
HEADS = 16
HEAD_DIM = 64
THETA = 10000.0
TRAIN_LEN = 2048
SCALE = 4.0
ALPHA = 1.0
BETA = 32.0
B, T = 2, 2048

NCORES = 8
HPC = HEADS // NCORES  # heads per core = 2

F32 = mybir.dt.float32
F32R = mybir.dt.float32r
BF16 = mybir.dt.bfloat16
BF = ml_dtypes.bfloat16

NEG_BIG = -1.0e30
DENOM_EPS = 1.0e-30

NT = T // 128   # 16 t-tiles of 128
NC4 = T // 512  # 4 chunks of 512
ND = HIDDEN // 128  # 8 d-chunks

# stream_shuffle mask: swap adjacent partitions within each 32-block
XMASK = [i ^ 1 for i in range(32)]
# pair-interleave permutation of the 64-dim head space
PERM64 = np.empty(64, dtype=np.int64)
PERM64[0::2] = np.arange(32)
PERM64[1::2] = np.arange(32, 64)


def _yarn_inv_freq_and_mscale():
    half = HEAD_DIM // 2
    pos_freqs = THETA ** (np.arange(half, dtype=np.float32) * 2.0 / HEAD_DIM)
    inv_freq_extra = (1.0 / pos_freqs).astype(np.float32)
    inv_freq_inter = (1.0 / (SCALE * pos_freqs)).astype(np.float32)

    def find_dim(num_rot):
        return (HEAD_DIM * math.log(TRAIN_LEN / (num_rot * 2.0 * math.pi))) / (
            2.0 * math.log(THETA)
        )

    low = max(math.floor(find_dim(BETA)), 0)
    high = min(math.ceil(find_dim(ALPHA)), half - 1)
    ramp = np.clip(
        (np.arange(half, dtype=np.float32) - low) / max(high - low, 1e-3), 0.0, 1.0
    ).astype(np.float32)
    extrap = (1.0 - ramp).astype(np.float32)
    inv_freq = inv_freq_inter * (1.0 - extrap) + inv_freq_extra * extrap
    mscale = 0.1 * math.log(SCALE) + 1.0 if SCALE > 1.0 else 1.0
    return inv_freq.astype(np.float32), np.float32(mscale)


def _host_prep(inputs):
    x = np.asarray(inputs["packed_embeddings"], dtype=np.float32)
    pos = np.asarray(inputs["position_ids"])
    act = np.asarray(inputs["active_mask"])
    wq = np.asarray(inputs["q_proj"], dtype=np.float32)
    wk = np.asarray(inputs["k_proj"], dtype=np.float32)
    wv = np.asarray(inputs["v_proj"], dtype=np.float32)
    wo = np.asarray(inputs["o_proj"], dtype=np.float32)

    inv_freq, mscale = _yarn_inv_freq_and_mscale()
    scale = np.float32(mscale / math.sqrt(HEAD_DIM))

    ang = pos.astype(np.float32)[..., None] * inv_freq  # (B, L, T, 32)
    cos32 = np.cos(ang).astype(np.float32)
    sin32 = np.sin(ang).astype(np.float32)
    # pair-interleaved tables: row 2j and 2j+1 both use freq j
    cos64 = np.repeat(cos32, 2, axis=-1)  # (B, L, T, 64)
    sin64 = np.repeat(sin32, 2, axis=-1).copy()
    sin64[..., 0::2] *= -1.0  # rotate-half sign baked in
    # transposed layout [B, L, 128, T]: rows 0-63 q (scale folded), 64-127 k
    cosT = np.concatenate([cos64 * scale, cos64], axis=-1).transpose(0, 1, 3, 2)
    sinT = np.concatenate([sin64 * scale, sin64], axis=-1).transpose(0, 1, 3, 2)
    cosT = np.ascontiguousarray(cosT, dtype=np.float32)
    sinT = np.ascontiguousarray(sinT, dtype=np.float32)

    # permuted q/k weights, packed [q|k] along the output dim
    wqp = wq[:, :, PERM64]
    wkp = wk[:, :, PERM64]
    wqk = np.ascontiguousarray(np.concatenate([wqp, wkp], axis=-1)).astype(BF)
    wv = np.ascontiguousarray(wv).astype(BF)  # (L, 1024, 64)
    wo = np.ascontiguousarray(wo, dtype=np.float32)  # (L, 64, 1024)

    # X^T tiles: [B, L, ND, 128, T] bf16
    xb = x.astype(BF)

    actf = act.astype(np.float32)  # (B, L, T)
    actb = ((actf - 1.0) * (-NEG_BIG)).reshape(B, HEADS, NT, 128).transpose(0, 1, 3, 2)
    actb = np.ascontiguousarray(actb, dtype=np.float32)
    act01 = np.ascontiguousarray(
        actf.reshape(B, HEADS, NT, 128).transpose(0, 1, 3, 2), dtype=np.float32
    )  # [B, L, 128, NT]

    tri = np.ascontiguousarray(np.triu(np.ones((128, 128), dtype=np.float32))).astype(BF)
    vones = np.zeros((128, 2), dtype=np.float32)
    vones[:, 0] = 1.0
    vones = vones.astype(BF)
    return xb, cosT, sinT, wqk, wv, wo, actb, act01, tri, vones


def make_in_maps(inputs):
    (xb, cosT, sinT, wqk, wv, wo, actb, act01, tri, vones) = _host_prep(inputs)
    in_maps = []
    for c in range(NCORES):
        hs = slice(c * HPC, (c + 1) * HPC)
        xs = xb[:, hs]  # [B, HPC, T, H]
        xt = np.ascontiguousarray(xs.transpose(0, 1, 3, 2)).reshape(
            B, HPC, ND, 128, T
        )
        in_maps.append({
            "xt": xt,
            "cos": np.ascontiguousarray(cosT[:, hs]),
            "sin": np.ascontiguousarray(sinT[:, hs]),
            "wqk": np.ascontiguousarray(wqk[hs]),
            "wv": np.ascontiguousarray(wv[hs]),
            "wo": np.ascontiguousarray(wo[hs]),
            "actb": np.ascontiguousarray(actb[:, hs]),
            "act01": np.ascontiguousarray(act01[:, hs]),
            "tri": tri,
            "vones": vones,
        })
    return in_maps


def _build_program():
    nc = bacc.Bacc("TRN2", target_bir_lowering=False, debug=False)

    xt_d = nc.declare_dram_parameter("xt", [B, HPC, ND, 128, T], BF16, isOutput=False)
    cos_d = nc.declare_dram_parameter("cos", [B, HPC, 128, T], F32, isOutput=False)
    sin_d = nc.declare_dram_parameter("sin", [B, HPC, 128, T], F32, isOutput=False)
    wqk_d = nc.declare_dram_parameter("wqk", [HPC, HIDDEN, 128], BF16, isOutput=False)
    wv_d = nc.declare_dram_parameter("wv", [HPC, HIDDEN, HEAD_DIM], BF16, isOutput=False)
    wo_d = nc.declare_dram_parameter("wo", [HPC, HEAD_DIM, HIDDEN], F32R, isOutput=False)
    actb_d = nc.declare_dram_parameter("actb", [B, HPC, 128, NT], F32, isOutput=False)
    act01_d = nc.declare_dram_parameter("act01", [B, HPC, 128, NT], F32, isOutput=False)
    tri_d = nc.declare_dram_parameter("tri", [128, 128], BF16, isOutput=False)
    vones_d = nc.declare_dram_parameter("vones", [128, 2], BF16, isOutput=False)
    out_d = nc.declare_dram_parameter("out", [B, HPC, T, HIDDEN], BF16, isOutput=True)

    with ExitStack() as ctx:
        tc = ctx.enter_context(tile.TileContext(nc))
        _emit(ctx, tc, nc, xt_d, cos_d, sin_d, wqk_d, wv_d, wo_d,
              actb_d, act01_d, tri_d, vones_d, out_d)
    nc.compile()
    return nc


def _emit(ctx, tc, nc, xt_d, cos_d, sin_d, wqk_d, wv_d, wo_d,
          actb_d, act01_d, tri_d, vones_d, out_d):
    # ---- pools ----
    consts = ctx.enter_context(tc.tile_pool(name="consts", bufs=1))
    wpool = ctx.enter_context(tc.tile_pool(name="wpool", bufs=2))
    xtp = ctx.enter_context(tc.tile_pool(name="xt", bufs=16))
    cssp = ctx.enter_context(tc.tile_pool(name="css", bufs=2))
    abp = ctx.enter_context(tc.tile_pool(name="ab", bufs=2))
    qkp = ctx.enter_context(tc.tile_pool(name="qk", bufs=2))
    krsp = ctx.enter_context(tc.tile_pool(name="krs", bufs=2))
    ropep = ctx.enter_context(tc.tile_pool(name="rope", bufs=2))
    vnp = ctx.enter_context(tc.tile_pool(name="vn", bufs=2))
    probp = ctx.enter_context(tc.tile_pool(name="prob", bufs=6))
    attp = ctx.enter_context(tc.tile_pool(name="att", bufs=2))
    rap = ctx.enter_context(tc.tile_pool(name="ra", bufs=2))
    outp = ctx.enter_context(tc.tile_pool(name="outsb", bufs=2))

    psum_pj = ctx.enter_context(tc.tile_pool(name="psum_pj", bufs=1, space="PSUM"))
    psum_sc = ctx.enter_context(tc.tile_pool(name="psum_sc", bufs=3, space="PSUM"))
    psum_att = ctx.enter_context(tc.tile_pool(name="psum_att", bufs=1, space="PSUM"))
    psum_dn = ctx.enter_context(tc.tile_pool(name="psum_dn", bufs=1, space="PSUM"))
    psum_o = ctx.enter_context(tc.tile_pool(name="psum_o", bufs=2, space="PSUM"))

    # ---- constants (once) ----
    tri_sb = consts.tile([128, 128], BF16)
    nc.sync.dma_start(out=tri_sb, in_=tri_d[:, :])
    vones_sb = consts.tile([128, 2], BF16)
    nc.sync.dma_start(out=vones_sb, in_=vones_d[:, :])
    ones_sb = consts.tile([128, 1], F32)
    nc.vector.memset(ones_sb, 1.0)

    pairs = [(b, h) for b in range(B) for h in range(HPC)]
    n_pairs = len(pairs)
    st = {}       # per-pair-idx state
    pending = []  # deferred chunk finishers

    def MM(label, *a, **kw):
        _MM_LABELS.append(label)
        nc.tensor.matmul(*a, **kw)

    # ---------- phase emitters ----------
    def emit_tables(idx):
        b, h = pairs[idx]
        s = st[idx] = {}
        t_qk = wpool.tile([128, ND, 128], BF16, tag="wqk", name="t_qk")
        nc.sync.dma_start(out=t_qk, in_=wqk_d[h].rearrange("(c p) m -> p c m", p=128))
        t_v = wpool.tile([128, ND, HEAD_DIM], BF16, tag="wv", name="t_v")
        nc.sync.dma_start(out=t_v, in_=wv_d[h].rearrange("(c p) m -> p c m", p=128))
        s["xt"] = []
        for dc in range(ND):
            xn = xtp.tile([128, T], BF16, tag="xt", name="xt")
            nc.sync.dma_start(out=xn, in_=xt_d[b, h, dc])
            s["xt"].append(xn)
        s["cos"] = cssp.tile([128, T], F32, tag="cos", name="cos_sb")
        nc.sync.dma_start(out=s["cos"], in_=cos_d[b, h])
        s["sin"] = cssp.tile([128, T], F32, tag="sin", name="sin_sb")
        nc.sync.dma_start(out=s["sin"], in_=sin_d[b, h])
        s["actb"] = abp.tile([128, NT], F32, tag="actb", name="actb_sb")
        nc.sync.dma_start(out=s["actb"], in_=actb_d[b, h])
        s["act01"] = abp.tile([128, NT], F32, tag="act01", name="act01_sb")
        nc.sync.dma_start(out=s["act01"], in_=act01_d[b, h])
        t_o = wpool.tile([HEAD_DIM, HIDDEN], F32R, tag="wo", name="t_o")
        nc.sync.dma_start(out=t_o, in_=wo_d[h])
        s["wqk"], s["wv"], s["wo"] = t_qk, t_v, t_o

    def proj_plan(idx):
        # projections + RoPE + v as a list of (marker, emit_fn) steps
        s = st[idx]

        def start_fn():
            qkr = qkp.tile([128, T], F32R, tag="qkr", name="qkr")
            kq = krsp.tile([64, T], F32R, tag="kq", name="kq")
            s["qkr"], s["kq"] = qkr, kq
            vn = vnp.tile([128, NT, HEAD_DIM + 2], BF16, tag="vn", name="vn")
            s["vn"] = vn
            vones_bcast = bass.AP(
                tensor=vones_sb.tensor,
                offset=vones_sb.offset,
                ap=[vones_sb.ap[0], [0, NT], vones_sb.ap[1]],
            )
            nc.sync.dma_start(out=vn[:, :, HEAD_DIM:HEAD_DIM + 2], in_=vones_bcast)

        plan = [("start", start_fn)]

        def qk_steps(ncx):
            tsl = slice(ncx * 512, (ncx + 1) * 512)
            box = {}

            def mm_fn():
                pq = box["pq"] = psum_pj.tile([128, 512], F32, tag="pj", name="pq")
                for dc in range(ND):
                    MM(f"p{idx}c{ncx}proj", pq, lhsT=s["wqk"][:, dc, :],
                       rhs=s["xt"][dc][:, tsl],
                       start=(dc == 0), stop=(dc == ND - 1))

            def evict_fn():
                box["qkn"] = ropep.tile([128, 512], F32, tag="qkn", name="qkn")
                nc.vector.tensor_copy(box["qkn"], box["pq"])

            def shuffle_fn():
                box["qksh"] = ropep.tile([128, 512], F32, tag="qksh", name="qksh")
                nc.vector.stream_shuffle(box["qksh"], box["qkn"], XMASK)

            def mulc_fn():
                box["qkc"] = ropep.tile([128, 512], F32, tag="qkc", name="qkc")
                nc.gpsimd.tensor_mul(box["qkc"], box["qkn"], s["cos"][:, tsl])

            def muls_fn():
                box["qks"] = ropep.tile([128, 512], F32, tag="qks", name="qks")
                nc.gpsimd.tensor_mul(box["qks"], box["qksh"], s["sin"][:, tsl])

            def add_fn():
                nc.gpsimd.tensor_add(s["qkr"][:, tsl], box["qkc"], box["qks"])

            def kq_fn():
                nc.scalar.dma_start(out=s["kq"][:, tsl], in_=s["qkr"][64:128, tsl])

            return [("c", mm_fn), ("c", evict_fn), ("c", shuffle_fn),
                    ("c", mulc_fn), ("c", muls_fn), ("c", add_fn),
                    (f"c{ncx}", kq_fn)]

        def v_steps(vg):
            box = {}
            steps = []

            def pv_fn():
                box["pv"] = psum_pj.tile([128, 512], F32, tag="pj", name="pv")

            def si_fn(k):
                si = vg * 8 + k
                ssl = slice(si * 128, (si + 1) * 128)

                def f():
                    for dc in range(ND):
                        MM(f"p{idx}v{si}", box["pv"][:, k * 64:(k + 1) * 64],
                           lhsT=s["xt"][dc][:, ssl], rhs=s["wv"][:, dc, :],
                           start=(dc == 0), stop=(dc == ND - 1),
                           skip_group_check=True)
                return f

            steps.append(("v", pv_fn))
            for k in range(8):
                steps.append(("v", si_fn(k)))

            def evict_fn():
                nc.vector.tensor_copy(
                    s["vn"][:, vg * 8:(vg + 1) * 8, 0:HEAD_DIM], box["pv"])

            steps.append((f"v{vg}", evict_fn))
            return steps

        plan += qk_steps(0) + v_steps(0) + qk_steps(1) + v_steps(1)
        plan += qk_steps(2) + qk_steps(3)
        return plan

    def make_finisher(idx, tcx, att_sb):
        b, h = pairs[idx]
        s = st[idx]
        wo = s["wo"]

        def fin():
            pdn = psum_dn.tile([128, 4], F32, tag="dn", name="pdn")
            for k in range(4):
                _MM_LABELS.append(f"p{idx}t{tcx}dntp{k}")
                nc.tensor.transpose(
                    out=pdn[:, k:k + 1],
                    in_=att_sb[HEAD_DIM:HEAD_DIM + 1,
                               k * 128:(k + 1) * 128].bitcast(F32),
                    identity=ones_sb[HEAD_DIM:HEAD_DIM + 1, :],
                )
            ra = rap.tile([128, 4], F32, tag="ra", name="ra")
            nc.vector.tensor_scalar_add(ra, pdn, DENOM_EPS)
            nc.vector.reciprocal(ra, ra)
            nc.vector.tensor_mul(ra, ra, s["act01"][:, tcx * 4:tcx * 4 + 4])
            osb = outp.tile([128, 4, HIDDEN], BF16, tag="osb", name="osb")
            for k in range(4):
                for dh in range(2):
                    po = psum_o.tile([128, 512], F32, tag="o", name="po")
                    MM(f"p{idx}t{tcx}o{k}{dh}",
                       po,
                       lhsT=att_sb[0:HEAD_DIM, k * 128:(k + 1) * 128],
                       rhs=wo[:, dh * 512:(dh + 1) * 512],
                       start=True, stop=True)
                    dst = osb[:, k, dh * 512:(dh + 1) * 512]
                    if (k * 2 + dh) % 2 == 1:
                        nc.scalar.mul(dst, po, ra[:, k:k + 1])
                    else:
                        nc.vector.tensor_scalar_mul(dst, po, ra[:, k:k + 1])
            nc.sync.dma_start(
                out=out_d[b, h, tcx * 512:(tcx + 1) * 512, :].rearrange(
                    "(k p) d -> p k d", k=4),
                in_=osb)
        return fin

    def emit_c_chunk(idx, tcx, filler=None):
        s = st[idx]
        qkr, kq, vn = s["qkr"], s["kq"], s["vn"]
        n_s = 4 * (tcx + 1)
        patt = psum_att.tile([HEAD_DIM + 2, 512], F32, tag="att", name="patt")
        pts = []
        offs = []

        def att_mm(si):
            off = offs[si]
            MM(f"p{idx}t{tcx}s{si}att", patt[:, off:], lhsT=vn[:, si, :],
               rhs=pts[si][:, off:],
               start=(si == 0), stop=(si == n_s - 1),
               skip_group_check=True)

        for si in range(n_s):
            kd = si - 4 * tcx
            off = max(kd, 0) * 128
            # f32r matmuls under 256 cols pay a 4x penalty; widen the tail
            offm = min(off, 256)
            tslm = slice(tcx * 512 + offm, (tcx + 1) * 512)
            psc = psum_sc.tile([128, 512], F32, tag="sc", name="psc")
            MM(f"p{idx}t{tcx}s{si}sc",
               psc[:, offm:],
               lhsT=kq[:, si * 128:(si + 1) * 128],
               rhs=qkr[0:64, tslm],
               start=True, stop=True)
            pt = probp.tile([128, 512], BF16, tag="prob", name="pt")
            nc.scalar.activation(pt[:, off:], psc[:, off:],
                                 mybir.ActivationFunctionType.Exp,
                                 bias=s["actb"][:, si:si + 1])
            if kd >= 0:
                nc.gpsimd.tensor_mul(pt[:, off:off + 128], pt[:, off:off + 128],
                                     tri_sb)
            pts.append(pt)
            offs.append(off)
            if filler is not None:
                filler()
            if si >= 3:
                att_mm(si - 3)
        att_mm(n_s - 3)
        att_mm(n_s - 2)
        att_mm(n_s - 1)
        att_sb = attp.tile([HEAD_DIM + 2, 512], F32R, tag="attsb", name="att_sb")
        nc.vector.tensor_copy(att_sb, patt)
        if pending:
            pending.pop(0)()
        pending.append(make_finisher(idx, tcx, att_sb))

    # ---------- interleaved pipeline across pairs ----------
    plans = {}   # idx -> [steps, pos]

    reached = {}  # idx -> set of markers already executed

    def step_one(idx):
        if idx not in plans:
            return False
        steps, pos = plans[idx]
        if pos >= len(steps):
            return False
        tag, fn = steps[pos]
        fn()
        reached.setdefault(idx, set()).add(tag)
        plans[idx][1] = pos + 1
        return True

    def drain_until(idx, marker):
        if idx not in plans:
            return
        while marker not in reached.setdefault(idx, set()):
            if not step_one(idx):
                return

    emit_tables(0)
    plans[0] = [proj_plan(0), 0]
    for idx in range(n_pairs):
        drain_until(idx, "v0")

        def filler():
            step_one(idx) or step_one(idx + 1)

        for tcx in range(NC4):
            if tcx >= 1:
                drain_until(idx, f"c{tcx}")
            if idx + 1 < n_pairs and tcx == 0:
                emit_tables(idx + 1)
                plans[idx + 1] = [proj_plan(idx + 1), 0]
            emit_c_chunk(idx, tcx, filler)
        drain_until(idx, "c3")
        if idx + 1 < n_pairs:
            drain_until(idx + 1, "c3")
        if idx > 0:
            del st[idx - 1]
            del plans[idx - 1]
    while pending:
        pending.pop(0)()


_PROGRAM = None
_MM_LABELS = []


def kernel(**inputs) -> np.ndarray:
    global _PROGRAM
    in_maps = make_in_maps(inputs)
    if _PROGRAM is None:
        _PROGRAM = _build_program()
    nc = _PROGRAM
    res = run_bass_kernel_spmd(nc, in_maps, list(range(NCORES)))
    outs = [np.asarray(res.results[c]["out"]).astype(np.float32)
            for c in range(NCORES)]
    return np.concatenate(outs, axis=1)


# revision 20
# speedup vs baseline: 1.0197x; 1.0197x over previous
"""Trainium2 Bass kernel for BottleneckedEnsembleAttention.

Sharding: 8 cores, core c handles heads [2c, 2c+1] for both batches
(4 independent (b, head) attention problems per core).

All-bf16 datapath (fp32 PSUM accumulation, fp32 scalars). Host precomputes:
  - X^T per (b,h) in bf16 ([8, 128, T] d-major tiles) -- no on-device
    transposes needed.
  - a pair-interleaved permutation of the head dim (u' = 2j <-> q[j],
    2j+1 <-> q[j+32]) applied to Wq/Wk columns and the YaRN cos/sin tables.
    Scores are invariant under any fixed permutation of the contraction dim,
    and in this layout rotate-half becomes a within-32-partition swap
    (partner = u XOR 1) which the DVE stream_shuffle does in ONE op --
    eliminating the second (rotated-weight) projection matmul pass.
  - sin table carries the rotate-half signs; cos/sin q-rows carry the
    softmax scale.

Per (b, h) on-device pipeline:
  1. load X^T tiles (bf16), per 512-col chunk: psum = [Wq|Wk]^T X^T,
     evict bf16, stream_shuffle, RoPE mul/add (Pool + DVE), partition-swap
     DMA to build kq (k rows 0-63 / q rows 64-127).
  2. v in natural layout: [s-tile, 64] = xt_tile^T @ Wv accumulated over
     d-chunks directly in PSUM col-slices (8 s-tiles per bank), plus a
     ones-column for the softmax denominator.
  3. per t-chunk (512 cols): scores^T = k lhsT @ q rhs with causal tail
     matmuls on diagonal blocks (cols [kd*128:512] only), exp via ACT
     (bias = -1e30 for inactive s), triangle mask on the diagonal 128-block
     (Pool), att^T accumulated over s-tiles with matching tail widths,
     o_proj per t-tile, eviction scaled by active[t]/denom[t] (DVE/ACT),
     batched 4-tile output store (bf16).

Emission is software-pipelined: pair i+1's projection/RoPE/v steps are
interleaved into pair i's attention chunks as PE gap filler; each chunk's
o_proj finisher is deferred one chunk.
"""

import math
from contextlib import ExitStack

import numpy as np
import ml_dtypes

import concourse.bass as bass
import concourse.mybir as mybir
import concourse.tile as tile
from concourse import bacc
from concourse.bass_utils import run_bass_kernel_spmd

# model constants (must match reference.py)
HIDDEN = 1024
HEADS = 16
HEAD_DIM = 64
THETA = 10000.0
TRAIN_LEN = 2048
SCALE = 4.0
ALPHA = 1.0
BETA = 32.0
B, T = 2, 2048

NCORES = 8
HPC = HEADS // NCORES  # heads per core = 2

F32 = mybir.dt.float32
F32R = mybir.dt.float32r
BF16 = mybir.dt.bfloat16
BF = ml_dtypes.bfloat16

NEG_BIG = -1.0e30
DENOM_EPS = 1.0e-30

NT = T // 128   # 16 t-tiles of 128
NC4 = T // 512  # 4 chunks of 512
ND = HIDDEN // 128  # 8 d-chunks

# stream_shuffle mask: swap adjacent partitions within each 32-block
XMASK = [i ^ 1 for i in range(32)]
# pair-interleave permutation of the 64-dim head space
PERM64 = np.empty(64, dtype=np.int64)
PERM64[0::2] = np.arange(32)
PERM64[1::2] = np.arange(32, 64)


def _yarn_inv_freq_and_mscale():
    half = HEAD_DIM // 2
    pos_freqs = THETA ** (np.arange(half, dtype=np.float32) * 2.0 / HEAD_DIM)
    inv_freq_extra = (1.0 / pos_freqs).astype(np.float32)
    inv_freq_inter = (1.0 / (SCALE * pos_freqs)).astype(np.float32)

    def find_dim(num_rot):
        return (HEAD_DIM * math.log(TRAIN_LEN / (num_rot * 2.0 * math.pi))) / (
            2.0 * math.log(THETA)
        )

    low = max(math.floor(find_dim(BETA)), 0)
    high = min(math.ceil(find_dim(ALPHA)), half - 1)
    ramp = np.clip(
        (np.arange(half, dtype=np.float32) - low) / max(high - low, 1e-3), 0.0, 1.0
    ).astype(np.float32)
    extrap = (1.0 - ramp).astype(np.float32)
    inv_freq = inv_freq_inter * (1.0 - extrap) + inv_freq_extra * extrap
    mscale = 0.1 * math.log(SCALE) + 1.0 if SCALE > 1.0 else 1.0
    return inv_freq.astype(np.float32), np.float32(mscale)


def _host_prep(inputs):
    x = np.asarray(inputs["packed_embeddings"], dtype=np.float32)
    pos = np.asarray(inputs["position_ids"])
    act = np.asarray(inputs["active_mask"])
    wq = np.asarray(inputs["q_proj"], dtype=np.float32)
    wk = np.asarray(inputs["k_proj"], dtype=np.float32)
    wv = np.asarray(inputs["v_proj"], dtype=np.float32)
    wo = np.asarray(inputs["o_proj"], dtype=np.float32)

    inv_freq, mscale = _yarn_inv_freq_and_mscale()
    scale = np.float32(mscale / math.sqrt(HEAD_DIM))

    ang = pos.astype(np.float32)[..., None] * inv_freq  # (B, L, T, 32)
    cos32 = np.cos(ang).astype(np.float32)
    sin32 = np.sin(ang).astype(np.float32)
    # pair-interleaved tables: row 2j and 2j+1 both use freq j
    cos64 = np.repeat(cos32, 2, axis=-1)  # (B, L, T, 64)
    sin64 = np.repeat(sin32, 2, axis=-1).copy()
    sin64[..., 0::2] *= -1.0  # rotate-half sign baked in
    # transposed layout [B, L, 128, T]: rows 0-63 q (scale folded), 64-127 k
    cosT = np.concatenate([cos64 * scale, cos64], axis=-1).transpose(0, 1, 3, 2)
    sinT = np.concatenate([sin64 * scale, sin64], axis=-1).transpose(0, 1, 3, 2)
    cosT = np.ascontiguousarray(cosT, dtype=np.float32)
    sinT = np.ascontiguousarray(sinT, dtype=np.float32)

    # permuted q/k weights, packed [q|k] along the output dim
    wqp = wq[:, :, PERM64]
    wkp = wk[:, :, PERM64]
    wqk = np.ascontiguousarray(np.concatenate([wqp, wkp], axis=-1)).astype(BF)
    wv = np.ascontiguousarray(wv).astype(BF)  # (L, 1024, 64)
    wo = np.ascontiguousarray(wo, dtype=np.float32)  # (L, 64, 1024)

    # X^T tiles: [B, L, ND, 128, T] bf16
    xb = x.astype(BF)

    actf = act.astype(np.float32)  # (B, L, T)
    actb = ((actf - 1.0) * (-NEG_BIG)).reshape(B, HEADS, NT, 128).transpose(0, 1, 3, 2)
    actb = np.ascontiguousarray(actb, dtype=np.float32)
    act01 = np.ascontiguousarray(
        actf.reshape(B, HEADS, NT, 128).transpose(0, 1, 3, 2), dtype=np.float32
    )  # [B, L, 128, NT]

    tri = np.ascontiguousarray(np.triu(np.ones((128, 128), dtype=np.float32))).astype(BF)
    vones = np.zeros((128, 2), dtype=np.float32)
    vones[:, 0] = 1.0
    vones = vones.astype(BF)
    return xb, cosT, sinT, wqk, wv, wo, actb, act01, tri, vones


def make_in_maps(inputs):
    (xb, cosT, sinT, wqk, wv, wo, actb, act01, tri, vones) = _host_prep(inputs)
    in_maps = []
    for c in range(NCORES):
        hs = slice(c * HPC, (c + 1) * HPC)
        xs = xb[:, hs]  # [B, HPC, T, H]
        xt = np.ascontiguousarray(xs.transpose(0, 1, 3, 2)).reshape(
            B, HPC, ND, 128, T
        )
        in_maps.append({
            "xt": xt,
            "cos": np.ascontiguousarray(cosT[:, hs]),
            "sin": np.ascontiguousarray(sinT[:, hs]),
            "wqk": np.ascontiguousarray(wqk[hs]),
            "wv": np.ascontiguousarray(wv[hs]),
            "wo": np.ascontiguousarray(wo[hs]),
            "actb": np.ascontiguousarray(actb[:, hs]),
            "act01": np.ascontiguousarray(act01[:, hs]),
            "tri": tri,
            "vones": vones,
        })
    return in_maps


def _build_program():
    nc = bacc.Bacc("TRN2", target_bir_lowering=False, debug=False)

    xt_d = nc.declare_dram_parameter("xt", [B, HPC, ND, 128, T], BF16, isOutput=False)
    cos_d = nc.declare_dram_parameter("cos", [B, HPC, 128, T], F32, isOutput=False)
    sin_d = nc.declare_dram_parameter("sin", [B, HPC, 128, T], F32, isOutput=False)
    wqk_d = nc.declare_dram_parameter("wqk", [HPC, HIDDEN, 128], BF16, isOutput=False)
    wv_d = nc.declare_dram_parameter("wv", [HPC, HIDDEN, HEAD_DIM], BF16, isOutput=False)
    wo_d = nc.declare_dram_parameter("wo", [HPC, HEAD_DIM, HIDDEN], F32R, isOutput=False)
    actb_d = nc.declare_dram_parameter("actb", [B, HPC, 128, NT], F32, isOutput=False)
    act01_d = nc.declare_dram_parameter("act01", [B, HPC, 128, NT], F32, isOutput=False)
    tri_d = nc.declare_dram_parameter("tri", [128, 128], BF16, isOutput=False)
    vones_d = nc.declare_dram_parameter("vones", [128, 2], BF16, isOutput=False)
    out_d = nc.declare_dram_parameter("out", [B, HPC, T, HIDDEN], BF16, isOutput=True)

    with ExitStack() as ctx:
        tc = ctx.enter_context(tile.TileContext(nc))
        _emit(ctx, tc, nc, xt_d, cos_d, sin_d, wqk_d, wv_d, wo_d,
              actb_d, act01_d, tri_d, vones_d, out_d)
    nc.compile()
    return nc


def _emit(ctx, tc, nc, xt_d, cos_d, sin_d, wqk_d, wv_d, wo_d,
          actb_d, act01_d, tri_d, vones_d, out_d):
    # ---- pools ----
    consts = ctx.enter_context(tc.tile_pool(name="consts", bufs=1))
    wpool = ctx.enter_context(tc.tile_pool(name="wpool", bufs=2))
    xtp = ctx.enter_context(tc.tile_pool(name="xt", bufs=16))
    cssp = ctx.enter_context(tc.tile_pool(name="css", bufs=2))
    abp = ctx.enter_context(tc.tile_pool(name="ab", bufs=2))
    qkp = ctx.enter_context(tc.tile_pool(name="qk", bufs=2))
    krsp = ctx.enter_context(tc.tile_pool(name="krs", bufs=2))
    ropep = ctx.enter_context(tc.tile_pool(name="rope", bufs=2))
    vnp = ctx.enter_context(tc.tile_pool(name="vn", bufs=2))
    probp = ctx.enter_context(tc.tile_pool(name="prob", bufs=6))
    attp = ctx.enter_context(tc.tile_pool(name="att", bufs=2))
    rap = ctx.enter_context(tc.tile_pool(name="ra", bufs=2))
    outp = ctx.enter_context(tc.tile_pool(name="outsb", bufs=2))

    psum_pj = ctx.enter_context(tc.tile_pool(name="psum_pj", bufs=2, space="PSUM"))
    psum_sc = ctx.enter_context(tc.tile_pool(name="psum_sc", bufs=2, space="PSUM"))
    psum_att = ctx.enter_context(tc.tile_pool(name="psum_att", bufs=1, space="PSUM"))
    psum_dn = ctx.enter_context(tc.tile_pool(name="psum_dn", bufs=1, space="PSUM"))
    psum_o = ctx.enter_context(tc.tile_pool(name="psum_o", bufs=2, space="PSUM"))

    # ---- constants (once) ----
    tri_sb = consts.tile([128, 128], BF16)
    nc.sync.dma_start(out=tri_sb, in_=tri_d[:, :])
    vones_sb = consts.tile([128, 2], BF16)
    nc.sync.dma_start(out=vones_sb, in_=vones_d[:, :])
    ones_sb = consts.tile([128, 1], F32)
    nc.vector.memset(ones_sb, 1.0)

    pairs = [(b, h) for b in range(B) for h in range(HPC)]
    n_pairs = len(pairs)
    st = {}       # per-pair-idx state
    pending = []  # deferred chunk finishers

    def MM(label, *a, **kw):
        _MM_LABELS.append(label)
        nc.tensor.matmul(*a, **kw)

    # ---------- phase emitters ----------
    def emit_tables(idx):
        b, h = pairs[idx]
        s = st[idx] = {}
        t_qk = wpool.tile([128, ND, 128], BF16, tag="wqk", name="t_qk")
        nc.sync.dma_start(out=t_qk, in_=wqk_d[h].rearrange("(c p) m -> p c m", p=128))
        t_v = wpool.tile([128, ND, HEAD_DIM], BF16, tag="wv", name="t_v")
        nc.sync.dma_start(out=t_v, in_=wv_d[h].rearrange("(c p) m -> p c m", p=128))
        s["actb"] = abp.tile([128, NT], F32, tag="actb", name="actb_sb")
        nc.sync.dma_start(out=s["actb"], in_=actb_d[b, h])
        s["act01"] = abp.tile([128, NT], F32, tag="act01", name="act01_sb")
        nc.sync.dma_start(out=s["act01"], in_=act01_d[b, h])
        s["xt"] = []
        for dc in range(ND):
            xn = xtp.tile([128, T], BF16, tag="xt", name="xt")
            nc.sync.dma_start(out=xn, in_=xt_d[b, h, dc])
            s["xt"].append(xn)
        s["cos"] = cssp.tile([128, T], F32, tag="cos", name="cos_sb")
        s["sin"] = cssp.tile([128, T], F32, tag="sin", name="sin_sb")
        for ncx in range(NC4):
            csl = slice(ncx * 512, (ncx + 1) * 512)
            nc.sync.dma_start(out=s["cos"][:, csl], in_=cos_d[b, h][:, csl])
            nc.sync.dma_start(out=s["sin"][:, csl], in_=sin_d[b, h][:, csl])
        t_o = wpool.tile([HEAD_DIM, HIDDEN], F32R, tag="wo", name="t_o")
        nc.sync.dma_start(out=t_o, in_=wo_d[h])
        s["wqk"], s["wv"], s["wo"] = t_qk, t_v, t_o

    def proj_plan(idx):
        # projections + RoPE + v as a list of (marker, emit_fn) steps
        s = st[idx]

        def start_fn():
            qkr = qkp.tile([128, T], F32R, tag="qkr", name="qkr")
            kq = krsp.tile([64, T], F32R, tag="kq", name="kq")
            s["qkr"], s["kq"] = qkr, kq
            vn = vnp.tile([128, NT, HEAD_DIM + 2], BF16, tag="vn", name="vn")
            s["vn"] = vn
            vones_bcast = bass.AP(
                tensor=vones_sb.tensor,
                offset=vones_sb.offset,
                ap=[vones_sb.ap[0], [0, NT], vones_sb.ap[1]],
            )
            nc.sync.dma_start(out=vn[:, :, HEAD_DIM:HEAD_DIM + 2], in_=vones_bcast)

        plan = [("start", start_fn)]

        def qk_steps(ncx):
            tsl = slice(ncx * 512, (ncx + 1) * 512)
            box = {}

            def mm_fn():
                pq = box["pq"] = psum_pj.tile([128, 512], F32, tag="pj", name="pq")
                for dc in range(ND):
                    MM(f"p{idx}c{ncx}proj", pq, lhsT=s["wqk"][:, dc, :],
                       rhs=s["xt"][dc][:, tsl],
                       start=(dc == 0), stop=(dc == ND - 1))

            def evict_fn():
                box["qkn"] = ropep.tile([128, 512], F32, tag="qkn", name="qkn")
                nc.vector.tensor_copy(box["qkn"], box["pq"])

            def shuffle_fn():
                box["qksh"] = ropep.tile([128, 512], F32, tag="qksh", name="qksh")
                nc.vector.stream_shuffle(box["qksh"], box["qkn"], XMASK)

            def mulc_fn():
                box["qkc"] = ropep.tile([128, 512], F32, tag="qkc", name="qkc")
                nc.gpsimd.tensor_mul(box["qkc"], box["qkn"], s["cos"][:, tsl])

            def muls_fn():
                box["qks"] = ropep.tile([128, 512], F32, tag="qks", name="qks")
                nc.gpsimd.tensor_mul(box["qks"], box["qksh"], s["sin"][:, tsl])

            def add_fn():
                nc.gpsimd.tensor_add(s["qkr"][:, tsl], box["qkc"], box["qks"])

            def kq_fn():
                nc.scalar.dma_start(out=s["kq"][:, tsl], in_=s["qkr"][64:128, tsl])

            return [("c", mm_fn), ("c", evict_fn), ("c", shuffle_fn),
                    ("c", mulc_fn), ("c", muls_fn), ("c", add_fn),
                    (f"c{ncx}", kq_fn)]

        def v_steps(vg):
            box = {}
            steps = []

            def pv_fn():
                box["pv"] = psum_pj.tile([128, 512], F32, tag="pj", name="pv")

            def si_fn(k):
                si = vg * 8 + k
                ssl = slice(si * 128, (si + 1) * 128)

                def f():
                    for dc in range(ND):
                        MM(f"p{idx}v{si}", box["pv"][:, k * 64:(k + 1) * 64],
                           lhsT=s["xt"][dc][:, ssl], rhs=s["wv"][:, dc, :],
                           start=(dc == 0), stop=(dc == ND - 1),
                           skip_group_check=True)
                return f

            steps.append(("v", pv_fn))
            for k in range(8):
                steps.append(("v", si_fn(k)))

            def evict_fn():
                nc.vector.tensor_copy(
                    s["vn"][:, vg * 8:(vg + 1) * 8, 0:HEAD_DIM], box["pv"])

            steps.append((f"v{vg}", evict_fn))
            return steps

        plan += qk_steps(0) + v_steps(0) + qk_steps(1) + v_steps(1)
        plan += qk_steps(2) + qk_steps(3)
        return plan

    def make_finisher(idx, tcx, att_sb):
        b, h = pairs[idx]
        s = st[idx]
        wo = s["wo"]

        def fin():
            pdn = psum_dn.tile([128, 4], F32, tag="dn", name="pdn")
            for k in range(4):
                _MM_LABELS.append(f"p{idx}t{tcx}dntp{k}")
                nc.tensor.transpose(
                    out=pdn[:, k:k + 1],
                    in_=att_sb[HEAD_DIM:HEAD_DIM + 1,
                               k * 128:(k + 1) * 128].bitcast(F32),
                    identity=ones_sb[HEAD_DIM:HEAD_DIM + 1, :],
                )
            ra = rap.tile([128, 4], F32, tag="ra", name="ra")
            nc.vector.tensor_scalar_add(ra, pdn, DENOM_EPS)
            nc.vector.reciprocal(ra, ra)
            nc.vector.tensor_mul(ra, ra, s["act01"][:, tcx * 4:tcx * 4 + 4])
            osb = outp.tile([128, 4, HIDDEN], BF16, tag="osb", name="osb")
            for k in range(4):
                for dh in range(2):
                    po = psum_o.tile([128, 512], F32, tag="o", name="po")
                    MM(f"p{idx}t{tcx}o{k}{dh}",
                       po,
                       lhsT=att_sb[0:HEAD_DIM, k * 128:(k + 1) * 128],
                       rhs=wo[:, dh * 512:(dh + 1) * 512],
                       start=True, stop=True)
                    dst = osb[:, k, dh * 512:(dh + 1) * 512]
                    if (k * 2 + dh) % 4 == 3:
                        nc.scalar.mul(dst, po, ra[:, k:k + 1])
                    else:
                        nc.vector.tensor_scalar_mul(dst, po, ra[:, k:k + 1])
            nc.sync.dma_start(
                out=out_d[b, h, tcx * 512:(tcx + 1) * 512, :].rearrange(
                    "(k p) d -> p k d", k=4),
                in_=osb)
        return fin

    def emit_c_chunk(idx, tcx, filler=None):
        s = st[idx]
        qkr, kq, vn = s["qkr"], s["kq"], s["vn"]
        n_s = 4 * (tcx + 1)
        patt = psum_att.tile([HEAD_DIM + 2, 512], F32, tag="att", name="patt")
        pts = []
        offs = []

        def att_mm(si):
            off = offs[si]
            MM(f"p{idx}t{tcx}s{si}att", patt[:, off:], lhsT=vn[:, si, :],
               rhs=pts[si][:, off:],
               start=(si == 0), stop=(si == n_s - 1),
               skip_group_check=True)

        for si in range(n_s):
            kd = si - 4 * tcx
            off = max(kd, 0) * 128
            # f32r matmuls under 256 cols pay a 4x penalty; widen the tail
            offm = min(off, 256)
            tslm = slice(tcx * 512 + offm, (tcx + 1) * 512)
            psc = psum_sc.tile([128, 512], F32, tag="sc", name="psc")
            MM(f"p{idx}t{tcx}s{si}sc",
               psc[:, offm:],
               lhsT=kq[:, si * 128:(si + 1) * 128],
               rhs=qkr[0:64, tslm],
               start=True, stop=True)
            pt = probp.tile([128, 512], BF16, tag="prob", name="pt")
            nc.scalar.activation(pt[:, off:], psc[:, off:],
                                 mybir.ActivationFunctionType.Exp,
                                 bias=s["actb"][:, si:si + 1])
            if kd >= 0:
                nc.gpsimd.tensor_mul(pt[:, off:off + 128], pt[:, off:off + 128],
                                     tri_sb)
            pts.append(pt)
            offs.append(off)
            if filler is not None:
                filler()
            if si >= 3:
                att_mm(si - 3)
        att_mm(n_s - 3)
        att_mm(n_s - 2)
        att_mm(n_s - 1)
        att_sb = attp.tile([HEAD_DIM + 2, 512], F32R, tag="attsb", name="att_sb")
        nc.vector.tensor_copy(att_sb, patt)
        if pending:
            pending.pop(0)()
        pending.append(make_finisher(idx, tcx, att_sb))

    # ---------- interleaved pipeline across pairs ----------
    plans = {}   # idx -> [steps, pos]

    reached = {}  # idx -> set of markers already executed

    def step_one(idx):
        if idx not in plans:
            return False
        steps, pos = plans[idx]
        if pos >= len(steps):
            return False
        tag, fn = steps[pos]
        fn()
        reached.setdefault(idx, set()).add(tag)
        plans[idx][1] = pos + 1
        return True

    def drain_until(idx, marker):
        if idx not in plans:
            return
        while marker not in reached.setdefault(idx, set()):
            if not step_one(idx):
                return

    emit_tables(0)
    plans[0] = [proj_plan(0), 0]
    for idx in range(n_pairs):
        drain_until(idx, "v0")

        def filler():
            step_one(idx) or step_one(idx + 1)

        for tcx in range(NC4):
            if tcx >= 1:
                drain_until(idx, f"c{tcx}")
            if idx + 1 < n_pairs and tcx == 0:
                emit_tables(idx + 1)
                plans[idx + 1] = [proj_plan(idx + 1), 0]
            emit_c_chunk(idx, tcx, filler)
        drain_until(idx, "c3")
        if idx + 1 < n_pairs:
            drain_until(idx + 1, "c3")
        if idx > 0:
            del st[idx - 1]
            del plans[idx - 1]
    while pending:
        pending.pop(0)()


_PROGRAM = None
_MM_LABELS = []


def kernel(**inputs) -> np.ndarray:
    global _PROGRAM
    in_maps = make_in_maps(inputs)
    if _PROGRAM is None:
        _PROGRAM = _build_program()
    nc = _PROGRAM
    res = run_bass_kernel_spmd(nc, in_maps, list(range(NCORES)))
    outs = [np.asarray(res.results[c]["out"]).astype(np.float32)
            for c in range(NCORES)]
    return np.concatenate(outs, axis=1)


# revision 21
# speedup vs baseline: 1.0630x; 1.0424x over previous
"""Trainium2 Bass kernel for BottleneckedEnsembleAttention.

Sharding: 8 cores, core c handles heads [2c, 2c+1] for both batches
(4 independent (b, head) attention problems per core).

All-bf16 datapath (fp32 PSUM accumulation, fp32 scalars). Host precomputes:
  - X^T per (b,h) in bf16 ([8, 128, T] d-major tiles) -- no on-device
    transposes needed.
  - a pair-interleaved permutation of the head dim (u' = 2j <-> q[j],
    2j+1 <-> q[j+32]) applied to Wq/Wk columns and the YaRN cos/sin tables.
    Scores are invariant under any fixed permutation of the contraction dim,
    and in this layout rotate-half becomes a within-32-partition swap
    (partner = u XOR 1) which the DVE stream_shuffle does in ONE op --
    eliminating the second (rotated-weight) projection matmul pass.
  - sin table carries the rotate-half signs; cos/sin q-rows carry the
    softmax scale.

Per (b, h) on-device pipeline:
  1. load X^T tiles (bf16), per 512-col chunk: psum = [Wq|Wk]^T X^T,
     evict bf16, stream_shuffle, RoPE mul/add (Pool + DVE), partition-swap
     DMA to build kq (k rows 0-63 / q rows 64-127).
  2. v in natural layout: [s-tile, 64] = xt_tile^T @ Wv accumulated over
     d-chunks directly in PSUM col-slices (8 s-tiles per bank), plus a
     ones-column for the softmax denominator.
  3. per t-chunk (512 cols): scores^T = k lhsT @ q rhs with causal tail
     matmuls on diagonal blocks (cols [kd*128:512] only), exp via ACT
     (bias = -1e30 for inactive s), triangle mask on the diagonal 128-block
     (Pool), att^T accumulated over s-tiles with matching tail widths,
     o_proj per t-tile, eviction scaled by active[t]/denom[t] (DVE/ACT),
     batched 4-tile output store (bf16).

Emission is software-pipelined: pair i+1's projection/RoPE/v steps are
interleaved into pair i's attention chunks as PE gap filler; each chunk's
o_proj finisher is deferred one chunk.
"""

import math
from contextlib import ExitStack

import numpy as np
import ml_dtypes

import concourse.bass as bass
import concourse.mybir as mybir
import concourse.tile as tile
from concourse import bacc
from concourse.bass_utils import run_bass_kernel_spmd

# model constants (must match reference.py)
HIDDEN = 1024
HEADS = 16
HEAD_DIM = 64
THETA = 10000.0
TRAIN_LEN = 2048
SCALE = 4.0
ALPHA = 1.0
BETA = 32.0
B, T = 2, 2048

NCORES = 8
HPC = HEADS // NCORES  # heads per core = 2

F32 = mybir.dt.float32
F32R = mybir.dt.float32r
BF16 = mybir.dt.bfloat16
BF = ml_dtypes.bfloat16

NEG_BIG = -1.0e30
DENOM_EPS = 1.0e-30

NT = T // 128   # 16 t-tiles of 128
NC4 = T // 512  # 4 chunks of 512
ND = HIDDEN // 128  # 8 d-chunks

# stream_shuffle mask: swap adjacent partitions within each 32-block
XMASK = [i ^ 1 for i in range(32)]
# pair-interleave permutation of the 64-dim head space
PERM64 = np.empty(64, dtype=np.int64)
PERM64[0::2] = np.arange(32)
PERM64[1::2] = np.arange(32, 64)


def _yarn_inv_freq_and_mscale():
    half = HEAD_DIM // 2
    pos_freqs = THETA ** (np.arange(half, dtype=np.float32) * 2.0 / HEAD_DIM)
    inv_freq_extra = (1.0 / pos_freqs).astype(np.float32)
    inv_freq_inter = (1.0 / (SCALE * pos_freqs)).astype(np.float32)

    def find_dim(num_rot):
        return (HEAD_DIM * math.log(TRAIN_LEN / (num_rot * 2.0 * math.pi))) / (
            2.0 * math.log(THETA)
        )

    low = max(math.floor(find_dim(BETA)), 0)
    high = min(math.ceil(find_dim(ALPHA)), half - 1)
    ramp = np.clip(
        (np.arange(half, dtype=np.float32) - low) / max(high - low, 1e-3), 0.0, 1.0
    ).astype(np.float32)
    extrap = (1.0 - ramp).astype(np.float32)
    inv_freq = inv_freq_inter * (1.0 - extrap) + inv_freq_extra * extrap
    mscale = 0.1 * math.log(SCALE) + 1.0 if SCALE > 1.0 else 1.0
    return inv_freq.astype(np.float32), np.float32(mscale)


def _host_prep(inputs):
    x = np.asarray(inputs["packed_embeddings"], dtype=np.float32)
    pos = np.asarray(inputs["position_ids"])
    act = np.asarray(inputs["active_mask"])
    wq = np.asarray(inputs["q_proj"], dtype=np.float32)
    wk = np.asarray(inputs["k_proj"], dtype=np.float32)
    wv = np.asarray(inputs["v_proj"], dtype=np.float32)
    wo = np.asarray(inputs["o_proj"], dtype=np.float32)

    inv_freq, mscale = _yarn_inv_freq_and_mscale()
    scale = np.float32(mscale / math.sqrt(HEAD_DIM))

    ang = pos.astype(np.float32)[..., None] * inv_freq  # (B, L, T, 32)
    cos32 = np.cos(ang).astype(np.float32)
    sin32 = np.sin(ang).astype(np.float32)
    # pair-interleaved tables: row 2j and 2j+1 both use freq j
    cos64 = np.repeat(cos32, 2, axis=-1)  # (B, L, T, 64)
    sin64 = np.repeat(sin32, 2, axis=-1).copy()
    sin64[..., 0::2] *= -1.0  # rotate-half sign baked in
    # transposed layout [B, L, 128, T]: rows 0-63 q (scale folded), 64-127 k
    cosT = np.concatenate([cos64 * scale, cos64], axis=-1).transpose(0, 1, 3, 2)
    sinT = np.concatenate([sin64 * scale, sin64], axis=-1).transpose(0, 1, 3, 2)
    cosT = np.ascontiguousarray(cosT, dtype=np.float32)
    sinT = np.ascontiguousarray(sinT, dtype=np.float32)

    # permuted q/k weights, packed [q|k] along the output dim
    wqp = wq[:, :, PERM64]
    wkp = wk[:, :, PERM64]
    wqk = np.ascontiguousarray(np.concatenate([wqp, wkp], axis=-1)).astype(BF)
    wv = np.ascontiguousarray(wv).astype(BF)  # (L, 1024, 64)
    wo = np.ascontiguousarray(wo, dtype=np.float32)  # (L, 64, 1024)

    # X^T tiles: [B, L, ND, 128, T] bf16
    xb = x.astype(BF)

    actf = act.astype(np.float32)  # (B, L, T)
    actb = ((actf - 1.0) * (-NEG_BIG)).reshape(B, HEADS, NT, 128).transpose(0, 1, 3, 2)
    actb = np.ascontiguousarray(actb, dtype=np.float32)
    act01 = np.ascontiguousarray(
        actf.reshape(B, HEADS, NT, 128).transpose(0, 1, 3, 2), dtype=np.float32
    )  # [B, L, 128, NT]

    tri = np.ascontiguousarray(np.triu(np.ones((128, 128), dtype=np.float32))).astype(BF)
    vones = np.zeros((128, 2), dtype=np.float32)
    vones[:, 0] = 1.0
    vones = vones.astype(BF)
    return xb, cosT, sinT, wqk, wv, wo, actb, act01, tri, vones


def make_in_maps(inputs):
    (xb, cosT, sinT, wqk, wv, wo, actb, act01, tri, vones) = _host_prep(inputs)
    in_maps = []
    for c in range(NCORES):
        hs = slice(c * HPC, (c + 1) * HPC)
        xs = xb[:, hs]  # [B, HPC, T, H]
        xt = np.ascontiguousarray(xs.transpose(0, 1, 3, 2)).reshape(
            B, HPC, ND, 128, T
        )
        in_maps.append({
            "xt": xt,
            "cos": np.ascontiguousarray(cosT[:, hs]),
            "sin": np.ascontiguousarray(sinT[:, hs]),
            "wqk": np.ascontiguousarray(wqk[hs]),
            "wv": np.ascontiguousarray(wv[hs]),
            "wo": np.ascontiguousarray(wo[hs]),
            "actb": np.ascontiguousarray(actb[:, hs]),
            "act01": np.ascontiguousarray(act01[:, hs]),
            "tri": tri,
            "vones": vones,
        })
    return in_maps


def _build_program():
    nc = bacc.Bacc("TRN2", target_bir_lowering=False, debug=False)

    xt_d = nc.declare_dram_parameter("xt", [B, HPC, ND, 128, T], BF16, isOutput=False)
    cos_d = nc.declare_dram_parameter("cos", [B, HPC, 128, T], F32, isOutput=False)
    sin_d = nc.declare_dram_parameter("sin", [B, HPC, 128, T], F32, isOutput=False)
    wqk_d = nc.declare_dram_parameter("wqk", [HPC, HIDDEN, 128], BF16, isOutput=False)
    wv_d = nc.declare_dram_parameter("wv", [HPC, HIDDEN, HEAD_DIM], BF16, isOutput=False)
    wo_d = nc.declare_dram_parameter("wo", [HPC, HEAD_DIM, HIDDEN], F32R, isOutput=False)
    actb_d = nc.declare_dram_parameter("actb", [B, HPC, 128, NT], F32, isOutput=False)
    act01_d = nc.declare_dram_parameter("act01", [B, HPC, 128, NT], F32, isOutput=False)
    tri_d = nc.declare_dram_parameter("tri", [128, 128], BF16, isOutput=False)
    vones_d = nc.declare_dram_parameter("vones", [128, 2], BF16, isOutput=False)
    out_d = nc.declare_dram_parameter("out", [B, HPC, T, HIDDEN], BF16, isOutput=True)

    with ExitStack() as ctx:
        tc = ctx.enter_context(tile.TileContext(nc))
        _emit(ctx, tc, nc, xt_d, cos_d, sin_d, wqk_d, wv_d, wo_d,
              actb_d, act01_d, tri_d, vones_d, out_d)
    nc.compile()
    return nc


def _emit(ctx, tc, nc, xt_d, cos_d, sin_d, wqk_d, wv_d, wo_d,
          actb_d, act01_d, tri_d, vones_d, out_d):
    # ---- pools ----
    consts = ctx.enter_context(tc.tile_pool(name="consts", bufs=1))
    wpool = ctx.enter_context(tc.tile_pool(name="wpool", bufs=2))
    xtp = ctx.enter_context(tc.tile_pool(name="xt", bufs=16))
    cssp = ctx.enter_context(tc.tile_pool(name="css", bufs=2))
    abp = ctx.enter_context(tc.tile_pool(name="ab", bufs=2))
    qkp = ctx.enter_context(tc.tile_pool(name="qk", bufs=2))
    krsp = ctx.enter_context(tc.tile_pool(name="krs", bufs=2))
    ropep = ctx.enter_context(tc.tile_pool(name="rope", bufs=2))
    vnp = ctx.enter_context(tc.tile_pool(name="vn", bufs=2))
    probp = ctx.enter_context(tc.tile_pool(name="prob", bufs=6))
    attp = ctx.enter_context(tc.tile_pool(name="att", bufs=2))
    rap = ctx.enter_context(tc.tile_pool(name="ra", bufs=2))
    outp = ctx.enter_context(tc.tile_pool(name="outsb", bufs=2))

    psum_pj = ctx.enter_context(tc.tile_pool(name="psum_pj", bufs=2, space="PSUM"))
    psum_sc = ctx.enter_context(tc.tile_pool(name="psum_sc", bufs=2, space="PSUM"))
    psum_att = ctx.enter_context(tc.tile_pool(name="psum_att", bufs=1, space="PSUM"))
    psum_dn = ctx.enter_context(tc.tile_pool(name="psum_dn", bufs=1, space="PSUM"))
    psum_o = ctx.enter_context(tc.tile_pool(name="psum_o", bufs=2, space="PSUM"))

    # ---- constants (once) ----
    tri_sb = consts.tile([128, 128], BF16)
    nc.sync.dma_start(out=tri_sb, in_=tri_d[:, :])
    vones_sb = consts.tile([128, 2], BF16)
    nc.sync.dma_start(out=vones_sb, in_=vones_d[:, :])
    ones_sb = consts.tile([128, 1], F32)
    nc.vector.memset(ones_sb, 1.0)

    pairs = [(b, h) for b in range(B) for h in range(HPC)]
    n_pairs = len(pairs)
    st = {}       # per-pair-idx state
    pending = []  # deferred chunk finishers

    def MM(label, *a, **kw):
        _MM_LABELS.append(label)
        nc.tensor.matmul(*a, **kw)

    # ---------- phase emitters ----------
    def emit_tables(idx):
        b, h = pairs[idx]
        s = st[idx] = {}
        t_qk = wpool.tile([128, ND, 128], BF16, tag="wqk", name="t_qk")
        nc.sync.dma_start(out=t_qk, in_=wqk_d[h].rearrange("(c p) m -> p c m", p=128))
        t_v = wpool.tile([128, ND, HEAD_DIM], BF16, tag="wv", name="t_v")
        nc.sync.dma_start(out=t_v, in_=wv_d[h].rearrange("(c p) m -> p c m", p=128))
        s["actb"] = abp.tile([128, NT], F32, tag="actb", name="actb_sb")
        nc.sync.dma_start(out=s["actb"], in_=actb_d[b, h])
        s["act01"] = abp.tile([128, NT], F32, tag="act01", name="act01_sb")
        nc.sync.dma_start(out=s["act01"], in_=act01_d[b, h])
        s["xt"] = []
        for dc in range(ND):
            xn = xtp.tile([128, T], BF16, tag="xt", name="xt")
            nc.sync.dma_start(out=xn, in_=xt_d[b, h, dc])
            s["xt"].append(xn)
        s["cos"] = cssp.tile([128, T], F32, tag="cos", name="cos_sb")
        s["sin"] = cssp.tile([128, T], F32, tag="sin", name="sin_sb")
        for ncx in range(NC4):
            csl = slice(ncx * 512, (ncx + 1) * 512)
            nc.sync.dma_start(out=s["cos"][:, csl], in_=cos_d[b, h][:, csl])
            nc.sync.dma_start(out=s["sin"][:, csl], in_=sin_d[b, h][:, csl])
        t_o = wpool.tile([HEAD_DIM, HIDDEN], F32R, tag="wo", name="t_o")
        nc.sync.dma_start(out=t_o, in_=wo_d[h])
        s["wqk"], s["wv"], s["wo"] = t_qk, t_v, t_o

    def proj_plan(idx):
        # projections + RoPE + v as a list of (marker, emit_fn) steps
        s = st[idx]

        def start_fn():
            qkr = qkp.tile([128, T], F32R, tag="qkr", name="qkr")
            kq = krsp.tile([64, T], F32R, tag="kq", name="kq")
            s["qkr"], s["kq"] = qkr, kq
            vn = vnp.tile([128, NT, HEAD_DIM + 2], BF16, tag="vn", name="vn")
            s["vn"] = vn
            vones_bcast = bass.AP(
                tensor=vones_sb.tensor,
                offset=vones_sb.offset,
                ap=[vones_sb.ap[0], [0, NT], vones_sb.ap[1]],
            )
            nc.sync.dma_start(out=vn[:, :, HEAD_DIM:HEAD_DIM + 2], in_=vones_bcast)

        plan = [("start", start_fn)]

        def qk_steps(ncx):
            tsl = slice(ncx * 512, (ncx + 1) * 512)
            box = {}

            def mm_fn():
                pq = box["pq"] = psum_pj.tile([128, 512], F32, tag="pj", name="pq")
                for dc in range(ND):
                    MM(f"p{idx}c{ncx}proj", pq, lhsT=s["wqk"][:, dc, :],
                       rhs=s["xt"][dc][:, tsl],
                       start=(dc == 0), stop=(dc == ND - 1))

            def evict_fn():
                box["qkn"] = ropep.tile([128, 512], F32, tag="qkn", name="qkn")
                nc.vector.tensor_copy(box["qkn"], box["pq"])

            def shuffle_fn():
                box["qksh"] = ropep.tile([128, 512], F32, tag="qksh", name="qksh")
                nc.vector.stream_shuffle(box["qksh"], box["qkn"], XMASK)

            def mulc_fn():
                box["qkc"] = ropep.tile([128, 512], F32, tag="qkc", name="qkc")
                nc.gpsimd.tensor_mul(box["qkc"], box["qkn"], s["cos"][:, tsl])

            def muls_fn():
                box["qks"] = ropep.tile([128, 512], F32, tag="qks", name="qks")
                nc.gpsimd.tensor_mul(box["qks"], box["qksh"], s["sin"][:, tsl])

            def add_fn():
                nc.gpsimd.tensor_add(s["qkr"][:, tsl], box["qkc"], box["qks"])

            def kq_fn():
                nc.scalar.dma_start(out=s["kq"][:, tsl], in_=s["qkr"][64:128, tsl])

            return [("c", mm_fn), ("c", evict_fn), ("c", shuffle_fn),
                    ("c", mulc_fn), ("c", muls_fn), ("c", add_fn),
                    (f"c{ncx}", kq_fn)]

        def v_steps(vg):
            box = {}
            steps = []

            def pv_fn():
                box["pv"] = psum_pj.tile([128, 512], F32, tag="pj", name="pv")

            def si_fn(k):
                si = vg * 8 + k
                ssl = slice(si * 128, (si + 1) * 128)

                def f():
                    for dc in range(ND):
                        MM(f"p{idx}v{si}", box["pv"][:, k * 64:(k + 1) * 64],
                           lhsT=s["xt"][dc][:, ssl], rhs=s["wv"][:, dc, :],
                           start=(dc == 0), stop=(dc == ND - 1),
                           skip_group_check=True)
                return f

            steps.append(("v", pv_fn))
            for k in range(8):
                steps.append(("v", si_fn(k)))

            def evict_fn():
                nc.vector.tensor_copy(
                    s["vn"][:, vg * 8:(vg + 1) * 8, 0:HEAD_DIM], box["pv"])

            steps.append((f"v{vg}", evict_fn))
            return steps

        plan += qk_steps(0) + qk_steps(1) + qk_steps(2) + qk_steps(3)
        plan += v_steps(0) + v_steps(1)
        return plan

    def make_finisher(idx, tcx, att_sb):
        b, h = pairs[idx]
        s = st[idx]
        wo = s["wo"]

        def fin():
            pdn = psum_dn.tile([128, 4], F32, tag="dn", name="pdn")
            for k in range(4):
                _MM_LABELS.append(f"p{idx}t{tcx}dntp{k}")
                nc.tensor.transpose(
                    out=pdn[:, k:k + 1],
                    in_=att_sb[HEAD_DIM:HEAD_DIM + 1,
                               k * 128:(k + 1) * 128].bitcast(F32),
                    identity=ones_sb[HEAD_DIM:HEAD_DIM + 1, :],
                )
            ra = rap.tile([128, 4], F32, tag="ra", name="ra")
            nc.vector.tensor_scalar_add(ra, pdn, DENOM_EPS)
            nc.vector.reciprocal(ra, ra)
            nc.vector.tensor_mul(ra, ra, s["act01"][:, tcx * 4:tcx * 4 + 4])
            osb = outp.tile([128, 4, HIDDEN], BF16, tag="osb", name="osb")
            for k in range(4):
                for dh in range(2):
                    po = psum_o.tile([128, 512], F32, tag="o", name="po")
                    MM(f"p{idx}t{tcx}o{k}{dh}",
                       po,
                       lhsT=att_sb[0:HEAD_DIM, k * 128:(k + 1) * 128],
                       rhs=wo[:, dh * 512:(dh + 1) * 512],
                       start=True, stop=True)
                    dst = osb[:, k, dh * 512:(dh + 1) * 512]
                    if (k * 2 + dh) % 4 == 3:
                        nc.scalar.mul(dst, po, ra[:, k:k + 1])
                    else:
                        nc.vector.tensor_scalar_mul(dst, po, ra[:, k:k + 1])
            nc.sync.dma_start(
                out=out_d[b, h, tcx * 512:(tcx + 1) * 512, :].rearrange(
                    "(k p) d -> p k d", k=4),
                in_=osb)
        return fin

    def emit_c_chunk(idx, tcx, filler=None):
        s = st[idx]
        qkr, kq, vn = s["qkr"], s["kq"], s["vn"]
        n_s = 4 * (tcx + 1)
        patt = psum_att.tile([HEAD_DIM + 2, 512], F32, tag="att", name="patt")
        pts = []
        offs = []

        def att_mm(si):
            off = offs[si]
            MM(f"p{idx}t{tcx}s{si}att", patt[:, off:], lhsT=vn[:, si, :],
               rhs=pts[si][:, off:],
               start=(si == 0), stop=(si == n_s - 1),
               skip_group_check=True)

        for si in range(n_s):
            kd = si - 4 * tcx
            off = max(kd, 0) * 128
            # f32r matmuls under 256 cols pay a 4x penalty; widen the tail
            offm = min(off, 256)
            tslm = slice(tcx * 512 + offm, (tcx + 1) * 512)
            psc = psum_sc.tile([128, 512], F32, tag="sc", name="psc")
            MM(f"p{idx}t{tcx}s{si}sc",
               psc[:, offm:],
               lhsT=kq[:, si * 128:(si + 1) * 128],
               rhs=qkr[0:64, tslm],
               start=True, stop=True)
            pt = probp.tile([128, 512], BF16, tag="prob", name="pt")
            nc.scalar.activation(pt[:, off:], psc[:, off:],
                                 mybir.ActivationFunctionType.Exp,
                                 bias=s["actb"][:, si:si + 1])
            if kd >= 0:
                nc.gpsimd.tensor_mul(pt[:, off:off + 128], pt[:, off:off + 128],
                                     tri_sb)
            pts.append(pt)
            offs.append(off)
            if filler is not None:
                filler()
            if si >= 3:
                att_mm(si - 3)
        att_mm(n_s - 3)
        att_mm(n_s - 2)
        att_mm(n_s - 1)
        att_sb = attp.tile([HEAD_DIM + 2, 512], F32R, tag="attsb", name="att_sb")
        nc.vector.tensor_copy(att_sb, patt)
        if pending:
            pending.pop(0)()
        pending.append(make_finisher(idx, tcx, att_sb))

    # ---------- interleaved pipeline across pairs ----------
    plans = {}   # idx -> [steps, pos]

    reached = {}  # idx -> set of markers already executed

    def step_one(idx):
        if idx not in plans:
            return False
        steps, pos = plans[idx]
        if pos >= len(steps):
            return False
        tag, fn = steps[pos]
        fn()
        reached.setdefault(idx, set()).add(tag)
        plans[idx][1] = pos + 1
        return True

    def drain_until(idx, marker):
        if idx not in plans:
            return
        while marker not in reached.setdefault(idx, set()):
            if not step_one(idx):
                return

    emit_tables(0)
    plans[0] = [proj_plan(0), 0]
    for _ in range(999):
        if not step_one(0):
            break
    FILLER_BUDGET = [0, 14, 32, 999]
    for idx in range(n_pairs):
        for tcx in range(NC4):
            if idx + 1 < n_pairs and tcx == 0:
                emit_tables(idx + 1)
                plans[idx + 1] = [proj_plan(idx + 1), 0]

            budget = FILLER_BUDGET[tcx]

            def filler():
                if idx + 1 in plans and plans[idx + 1][1] < budget:
                    step_one(idx + 1)

            emit_c_chunk(idx, tcx, filler)
        if idx + 1 < n_pairs:
            while step_one(idx + 1):
                pass
        if idx > 0:
            del st[idx - 1]
            del plans[idx - 1]
    while pending:
        pending.pop(0)()


_PROGRAM = None
_MM_LABELS = []


def kernel(**inputs) -> np.ndarray:
    global _PROGRAM
    in_maps = make_in_maps(inputs)
    if _PROGRAM is None:
        _PROGRAM = _build_program()
    nc = _PROGRAM
    res = run_bass_kernel_spmd(nc, in_maps, list(range(NCORES)))
    outs = [np.asarray(res.results[c]["out"]).astype(np.float32)
            for c in range(NCORES)]
    return np.concatenate(outs, axis=1)


# revision 33
# speedup vs baseline: 1.1875x; 1.1171x over previous
"""Trainium2 Bass kernel for BottleneckedEnsembleAttention.

Sharding: 8 cores, core c handles heads [2c, 2c+1] for both batches
(4 independent (b, head) attention problems per core).

All-bf16 datapath (fp32 PSUM accumulation, fp32 scalars). Host precomputes:
  - X^T per (b,h) in bf16 ([8, 128, T] d-major tiles) -- no on-device
    transposes needed.
  - a pair-interleaved permutation of the head dim (u' = 2j <-> q[j],
    2j+1 <-> q[j+32]) applied to Wq/Wk columns and the YaRN cos/sin tables.
    Scores are invariant under any fixed permutation of the contraction dim,
    and in this layout rotate-half becomes a within-32-partition swap
    (partner = u XOR 1) which the DVE stream_shuffle does in ONE op --
    eliminating the second (rotated-weight) projection matmul pass.
  - sin table carries the rotate-half signs; cos/sin q-rows carry the
    softmax scale.

Per (b, h) on-device pipeline:
  1. load X^T tiles (bf16), per 512-col chunk: psum = [Wq|Wk]^T X^T,
     evict bf16, stream_shuffle, RoPE mul/add (Pool + DVE), partition-swap
     DMA to build kq (k rows 0-63 / q rows 64-127).
  2. v in natural layout: [s-tile, 64] = xt_tile^T @ Wv accumulated over
     d-chunks directly in PSUM col-slices (8 s-tiles per bank), plus a
     ones-column for the softmax denominator.
  3. per t-chunk (512 cols): scores^T = k lhsT @ q rhs with causal tail
     matmuls on diagonal blocks (cols [kd*128:512] only), exp via ACT
     (bias = -1e30 for inactive s), triangle mask on the diagonal 128-block
     (Pool), att^T accumulated over s-tiles with matching tail widths,
     o_proj per t-tile, eviction scaled by active[t]/denom[t] (DVE/ACT),
     batched 4-tile output store (bf16).

Emission is software-pipelined: pair i+1's projection/RoPE/v steps are
interleaved into pair i's attention chunks as PE gap filler; each chunk's
o_proj finisher is deferred one chunk.
"""

import math
from contextlib import ExitStack

import numpy as np
import ml_dtypes

import concourse.bass as bass
import concourse.mybir as mybir
import concourse.tile as tile
from concourse import bacc
from concourse.bass_utils import run_bass_kernel_spmd

# model constants (must match reference.py)
HIDDEN = 1024
HEADS = 16
HEAD_DIM = 64
THETA = 10000.0
TRAIN_LEN = 2048
SCALE = 4.0
ALPHA = 1.0
BETA = 32.0
B, T = 2, 2048

NCORES = 8
HPC = HEADS // NCORES  # heads per core = 2

F32 = mybir.dt.float32
F32R = mybir.dt.float32r
BF16 = mybir.dt.bfloat16
BF = ml_dtypes.bfloat16

NEG_BIG = -1.0e30
DENOM_EPS = 1.0e-30

NT = T // 128   # 16 t-tiles of 128
NC4 = T // 512  # 4 chunks of 512
ND = HIDDEN // 128  # 8 d-chunks

# stream_shuffle mask: swap adjacent partitions within each 32-block
XMASK = [i ^ 1 for i in range(32)]
# pair-interleave permutation of the 64-dim head space
PERM64 = np.empty(64, dtype=np.int64)
PERM64[0::2] = np.arange(32)
PERM64[1::2] = np.arange(32, 64)


def _yarn_inv_freq_and_mscale():
    half = HEAD_DIM // 2
    pos_freqs = THETA ** (np.arange(half, dtype=np.float32) * 2.0 / HEAD_DIM)
    inv_freq_extra = (1.0 / pos_freqs).astype(np.float32)
    inv_freq_inter = (1.0 / (SCALE * pos_freqs)).astype(np.float32)

    def find_dim(num_rot):
        return (HEAD_DIM * math.log(TRAIN_LEN / (num_rot * 2.0 * math.pi))) / (
            2.0 * math.log(THETA)
        )

    low = max(math.floor(find_dim(BETA)), 0)
    high = min(math.ceil(find_dim(ALPHA)), half - 1)
    ramp = np.clip(
        (np.arange(half, dtype=np.float32) - low) / max(high - low, 1e-3), 0.0, 1.0
    ).astype(np.float32)
    extrap = (1.0 - ramp).astype(np.float32)
    inv_freq = inv_freq_inter * (1.0 - extrap) + inv_freq_extra * extrap
    mscale = 0.1 * math.log(SCALE) + 1.0 if SCALE > 1.0 else 1.0
    return inv_freq.astype(np.float32), np.float32(mscale)


def _host_prep(inputs):
    x = np.asarray(inputs["packed_embeddings"], dtype=np.float32)
    pos = np.asarray(inputs["position_ids"])
    act = np.asarray(inputs["active_mask"])
    wq = np.asarray(inputs["q_proj"], dtype=np.float32)
    wk = np.asarray(inputs["k_proj"], dtype=np.float32)
    wv = np.asarray(inputs["v_proj"], dtype=np.float32)
    wo = np.asarray(inputs["o_proj"], dtype=np.float32)

    inv_freq, mscale = _yarn_inv_freq_and_mscale()
    scale = np.float32(mscale / math.sqrt(HEAD_DIM))

    ang = pos.astype(np.float32)[..., None] * inv_freq  # (B, L, T, 32)
    cos32 = np.cos(ang).astype(np.float32)
    sin32 = np.sin(ang).astype(np.float32)
    # pair-interleaved tables: row 2j and 2j+1 both use freq j
    cos64 = np.repeat(cos32, 2, axis=-1)  # (B, L, T, 64)
    sin64 = np.repeat(sin32, 2, axis=-1).copy()
    sin64[..., 0::2] *= -1.0  # rotate-half sign baked in
    # transposed layout [B, L, 128, T]: rows 0-63 q (scale folded), 64-127 k
    cosT = np.concatenate([cos64 * scale, cos64], axis=-1).transpose(0, 1, 3, 2)
    sinT = np.concatenate([sin64 * scale, sin64], axis=-1).transpose(0, 1, 3, 2)
    cosT = np.ascontiguousarray(cosT).astype(BF)
    sinT = np.ascontiguousarray(sinT).astype(BF)

    # permuted q/k weights, packed [q|k] along the output dim
    wqp = wq[:, :, PERM64]
    wkp = wk[:, :, PERM64]
    wqk = np.ascontiguousarray(np.concatenate([wqp, wkp], axis=-1)).astype(BF)
    wv = np.ascontiguousarray(wv).astype(BF)  # (L, 1024, 64)
    wo = np.ascontiguousarray(wo, dtype=np.float32)  # (L, 64, 1024)

    # X^T tiles: [B, L, ND, 128, T] bf16
    xb = x.astype(BF)

    actf = act.astype(np.float32)  # (B, L, T)
    actb = ((actf - 1.0) * (-NEG_BIG)).reshape(B, HEADS, NT, 128).transpose(0, 1, 3, 2)
    actb = np.ascontiguousarray(actb, dtype=np.float32)
    act01 = np.ascontiguousarray(
        actf.reshape(B, HEADS, NT, 128).transpose(0, 1, 3, 2), dtype=np.float32
    )  # [B, L, 128, NT]

    tri = np.ascontiguousarray(np.triu(np.ones((128, 128), dtype=np.float32))).astype(BF)
    vones = np.zeros((128, 2), dtype=np.float32)
    vones[:, 0] = 1.0
    vones = vones.astype(BF)
    return xb, cosT, sinT, wqk, wv, wo, actb, act01, tri, vones


def make_in_maps(inputs):
    (xb, cosT, sinT, wqk, wv, wo, actb, act01, tri, vones) = _host_prep(inputs)
    in_maps = []
    for c in range(NCORES):
        hs = slice(c * HPC, (c + 1) * HPC)
        xs = xb[:, hs]  # [B, HPC, T, H]
        xt = np.ascontiguousarray(xs.transpose(0, 1, 3, 2)).reshape(
            B, HPC, ND, 128, T
        )
        in_maps.append({
            "xt": xt,
            "cos": np.ascontiguousarray(cosT[:, hs]),
            "sin": np.ascontiguousarray(sinT[:, hs]),
            "wqk": np.ascontiguousarray(wqk[hs]),
            "wv": np.ascontiguousarray(wv[hs]),
            "wo": np.ascontiguousarray(wo[hs]),
            "actb": np.ascontiguousarray(actb[:, hs]),
            "act01": np.ascontiguousarray(act01[:, hs]),
            "tri": tri,
            "vones": vones,
        })
    return in_maps


def _build_program():
    nc = bacc.Bacc("TRN2", target_bir_lowering=False, debug=False)

    xt_d = nc.declare_dram_parameter("xt", [B, HPC, ND, 128, T], BF16, isOutput=False)
    cos_d = nc.declare_dram_parameter("cos", [B, HPC, 128, T], BF16, isOutput=False)
    sin_d = nc.declare_dram_parameter("sin", [B, HPC, 128, T], BF16, isOutput=False)
    wqk_d = nc.declare_dram_parameter("wqk", [HPC, HIDDEN, 128], BF16, isOutput=False)
    wv_d = nc.declare_dram_parameter("wv", [HPC, HIDDEN, HEAD_DIM], BF16, isOutput=False)
    wo_d = nc.declare_dram_parameter("wo", [HPC, HEAD_DIM, HIDDEN], F32R, isOutput=False)
    actb_d = nc.declare_dram_parameter("actb", [B, HPC, 128, NT], F32, isOutput=False)
    act01_d = nc.declare_dram_parameter("act01", [B, HPC, 128, NT], F32, isOutput=False)
    tri_d = nc.declare_dram_parameter("tri", [128, 128], BF16, isOutput=False)
    vones_d = nc.declare_dram_parameter("vones", [128, 2], BF16, isOutput=False)
    out_d = nc.declare_dram_parameter("out", [B, HPC, T, HIDDEN], BF16, isOutput=True)

    with ExitStack() as ctx:
        tc = ctx.enter_context(tile.TileContext(nc))
        _emit(ctx, tc, nc, xt_d, cos_d, sin_d, wqk_d, wv_d, wo_d,
              actb_d, act01_d, tri_d, vones_d, out_d)
    nc.compile()
    return nc


def _emit(ctx, tc, nc, xt_d, cos_d, sin_d, wqk_d, wv_d, wo_d,
          actb_d, act01_d, tri_d, vones_d, out_d):
    # ---- pools ----
    consts = ctx.enter_context(tc.tile_pool(name="consts", bufs=1))
    wpool = ctx.enter_context(tc.tile_pool(name="wpool", bufs=2))
    xtp = ctx.enter_context(tc.tile_pool(name="xt", bufs=16))
    cssp = ctx.enter_context(tc.tile_pool(name="css", bufs=2))
    abp = ctx.enter_context(tc.tile_pool(name="ab", bufs=2))
    qkp = ctx.enter_context(tc.tile_pool(name="qk", bufs=2))
    krsp = ctx.enter_context(tc.tile_pool(name="krs", bufs=2))
    ropep = ctx.enter_context(tc.tile_pool(name="rope", bufs=2))
    vnp = ctx.enter_context(tc.tile_pool(name="vn", bufs=2))
    probp = ctx.enter_context(tc.tile_pool(name="prob", bufs=6))
    attp = ctx.enter_context(tc.tile_pool(name="att", bufs=2))
    rap = ctx.enter_context(tc.tile_pool(name="ra", bufs=2))
    outp = ctx.enter_context(tc.tile_pool(name="outsb", bufs=2))

    psum_pj = ctx.enter_context(tc.tile_pool(name="psum_pj", bufs=2, space="PSUM"))
    psum_sc = ctx.enter_context(tc.tile_pool(name="psum_sc", bufs=3, space="PSUM"))
    psum_att = ctx.enter_context(tc.tile_pool(name="psum_att", bufs=1, space="PSUM"))
    psum_o = ctx.enter_context(tc.tile_pool(name="psum_o", bufs=2, space="PSUM"))

    # ---- constants (once) ----
    tri_sb = consts.tile([128, 128], BF16)
    nc.sync.dma_start(out=tri_sb, in_=tri_d[:, :])
    vones_sb = consts.tile([128, 2], BF16)
    nc.sync.dma_start(out=vones_sb, in_=vones_d[:, :])
    ones_sb = consts.tile([128, 1], F32)
    nc.vector.memset(ones_sb, 1.0)

    pairs = [(b, h) for b in range(B) for h in range(HPC)]
    n_pairs = len(pairs)
    st = {}       # per-pair-idx state
    pending = []  # deferred chunk finishers

    def MM(label, *a, **kw):
        _MM_LABELS.append(label)
        nc.tensor.matmul(*a, **kw)

    # ---------- phase emitters ----------
    def emit_tables(idx):
        b, h = pairs[idx]
        s = st[idx] = {}
        t_qk = wpool.tile([128, ND, 128], BF16, tag="wqk", name="t_qk")
        nc.sync.dma_start(out=t_qk, in_=wqk_d[h].rearrange("(c p) m -> p c m", p=128))
        t_v = wpool.tile([128, ND, HEAD_DIM], BF16, tag="wv", name="t_v")
        nc.sync.dma_start(out=t_v, in_=wv_d[h].rearrange("(c p) m -> p c m", p=128))
        s["actb"] = abp.tile([128, NT], F32, tag="actb", name="actb_sb")
        nc.sync.dma_start(out=s["actb"], in_=actb_d[b, h])
        s["act01"] = abp.tile([128, NT], F32, tag="act01", name="act01_sb")
        nc.sync.dma_start(out=s["act01"], in_=act01_d[b, h])
        s["xt"] = []
        for dc in range(ND):
            xn = xtp.tile([128, T], BF16, tag="xt", name="xt")
            nc.sync.dma_start(out=xn, in_=xt_d[b, h, dc])
            s["xt"].append(xn)
        s["cos"] = cssp.tile([128, T], BF16, tag="cos", name="cos_sb")
        s["sin"] = cssp.tile([128, T], BF16, tag="sin", name="sin_sb")
        for ncx in range(NC4):
            csl = slice(ncx * 512, (ncx + 1) * 512)
            nc.sync.dma_start(out=s["cos"][:, csl], in_=cos_d[b, h][:, csl])
            nc.sync.dma_start(out=s["sin"][:, csl], in_=sin_d[b, h][:, csl])
        t_o = wpool.tile([HEAD_DIM, HIDDEN], F32R, tag="wo", name="t_o")
        nc.sync.dma_start(out=t_o, in_=wo_d[h])
        s["wqk"], s["wv"], s["wo"] = t_qk, t_v, t_o

    def proj_plan(idx, interleaved=False):
        # projections + RoPE + v as a list of (marker, emit_fn) steps
        s = st[idx]

        def start_fn():
            qkr = qkp.tile([128, T], F32R, tag="qkr", name="qkr")
            kq = krsp.tile([64, T], F32R, tag="kq", name="kq")
            s["qkr"], s["kq"] = qkr, kq
            vn = vnp.tile([128, NT, HEAD_DIM + 2], BF16, tag="vn", name="vn")
            s["vn"] = vn
            vones_bcast = bass.AP(
                tensor=vones_sb.tensor,
                offset=vones_sb.offset,
                ap=[vones_sb.ap[0], [0, NT], vones_sb.ap[1]],
            )
            nc.sync.dma_start(out=vn[:, :, HEAD_DIM:HEAD_DIM + 2], in_=vones_bcast)

        plan = [("start", start_fn)]

        def qk_steps(ncx):
            tsl = slice(ncx * 512, (ncx + 1) * 512)
            box = {}

            def mm_fn():
                pq = box["pq"] = psum_pj.tile([128, 512], F32, tag="pj", name="pq")
                for dc in range(ND):
                    MM(f"p{idx}c{ncx}proj", pq, lhsT=s["wqk"][:, dc, :],
                       rhs=s["xt"][dc][:, tsl],
                       start=(dc == 0), stop=(dc == ND - 1))

            def evict_fn():
                box["qkn"] = ropep.tile([128, 512], F32, tag="qkn", name="qkn")
                nc.vector.tensor_copy(box["qkn"], box["pq"])

            def shuffle_fn():
                box["qksh"] = ropep.tile([128, 512], F32, tag="qksh", name="qksh")
                nc.vector.stream_shuffle(box["qksh"], box["qkn"], XMASK)

            def mulc_fn():
                box["qkc"] = ropep.tile([128, 512], F32, tag="qkc", name="qkc")
                nc.gpsimd.tensor_mul(box["qkc"], box["qkn"], s["cos"][:, tsl])

            def muls_fn():
                box["qks"] = ropep.tile([128, 512], F32, tag="qks", name="qks")
                nc.gpsimd.tensor_mul(box["qks"], box["qksh"], s["sin"][:, tsl])

            def add_fn():
                nc.vector.tensor_add(s["qkr"][:, tsl], box["qkc"], box["qks"])

            def kq_fn():
                nc.scalar.dma_start(out=s["kq"][:, tsl], in_=s["qkr"][64:128, tsl])

            return [("c", mm_fn), ("c", evict_fn), ("c", shuffle_fn),
                    ("c", mulc_fn), ("c", muls_fn), ("c", add_fn),
                    (f"c{ncx}", kq_fn)]

        def v_steps(vg):
            box = {}
            steps = []

            def pv_fn():
                box["pv"] = psum_pj.tile([128, 512], F32, tag="pj", name="pv")

            def si_fn(k):
                si = vg * 8 + k
                ssl = slice(si * 128, (si + 1) * 128)

                def f():
                    for dc in range(ND):
                        MM(f"p{idx}v{si}", box["pv"][:, k * 64:(k + 1) * 64],
                           lhsT=s["xt"][dc][:, ssl], rhs=s["wv"][:, dc, :],
                           start=(dc == 0), stop=(dc == ND - 1),
                           skip_group_check=True)
                return f

            steps.append(("v", pv_fn))
            for k in range(8):
                steps.append(("v", si_fn(k)))

            def evict_fn():
                nc.vector.tensor_copy(
                    s["vn"][:, vg * 8:(vg + 1) * 8, 0:HEAD_DIM], box["pv"])

            steps.append((f"v{vg}", evict_fn))
            return steps

        if interleaved:
            plan += qk_steps(0) + v_steps(0) + qk_steps(1) + v_steps(1)
            plan += qk_steps(2) + qk_steps(3)
        else:
            plan += qk_steps(0) + qk_steps(1) + qk_steps(2) + qk_steps(3)
            plan += v_steps(0) + v_steps(1)
        return plan

    def make_finisher(idx, tcx, att_sb):
        b, h = pairs[idx]
        s = st[idx]
        wo = s["wo"]

        def fin():
            pdn_t = psum_sc.tile([128, 512], F32, tag="sc", name="pdn_t")
            pdn = pdn_t[:, 0:4]
            for k in range(4):
                _MM_LABELS.append(f"p{idx}t{tcx}dntp{k}")
                nc.tensor.transpose(
                    out=pdn[:, k:k + 1],
                    in_=att_sb[HEAD_DIM:HEAD_DIM + 1,
                               k * 128:(k + 1) * 128].bitcast(F32),
                    identity=ones_sb[HEAD_DIM:HEAD_DIM + 1, :],
                )
            ra = rap.tile([128, 4], F32, tag="ra", name="ra")
            nc.vector.tensor_scalar_add(ra, pdn, DENOM_EPS)
            nc.vector.reciprocal(ra, ra)
            nc.vector.tensor_mul(ra, ra, s["act01"][:, tcx * 4:tcx * 4 + 4])
            osb = outp.tile([128, 4, HIDDEN], BF16, tag="osb", name="osb")
            for k in range(4):
                for dh in range(2):
                    po = psum_o.tile([128, 512], F32, tag="o", name="po")
                    MM(f"p{idx}t{tcx}o{k}{dh}",
                       po,
                       lhsT=att_sb[0:HEAD_DIM, k * 128:(k + 1) * 128],
                       rhs=wo[:, dh * 512:(dh + 1) * 512],
                       start=True, stop=True)
                    dst = osb[:, k, dh * 512:(dh + 1) * 512]
                    if (k * 2 + dh) % 4 == 3:
                        nc.scalar.mul(dst, po, ra[:, k:k + 1])
                    else:
                        nc.vector.tensor_scalar_mul(dst, po, ra[:, k:k + 1])
            nc.sync.dma_start(
                out=out_d[b, h, tcx * 512:(tcx + 1) * 512, :].rearrange(
                    "(k p) d -> p k d", k=4),
                in_=osb)
        return fin

    def emit_c_chunk(idx, tcx, filler=None):
        s = st[idx]
        qkr, kq, vn = s["qkr"], s["kq"], s["vn"]
        n_s = 4 * (tcx + 1)
        patt = psum_att.tile([HEAD_DIM + 2, 512], F32, tag="att", name="patt")
        pts = []
        offs = []

        def att_mm(si):
            off = offs[si]
            MM(f"p{idx}t{tcx}s{si}att", patt[:, off:], lhsT=vn[:, si, :],
               rhs=pts[si][:, off:],
               start=(si == 0), stop=(si == n_s - 1),
               skip_group_check=True)

        for si in range(n_s):
            kd = si - 4 * tcx
            off = max(kd, 0) * 128
            # f32r matmuls under 256 cols pay a 4x penalty; widen the tail
            offm = min(off, 256)
            tslm = slice(tcx * 512 + offm, (tcx + 1) * 512)
            psc = psum_sc.tile([128, 512], F32, tag="sc", name="psc")
            MM(f"p{idx}t{tcx}s{si}sc",
               psc[:, offm:],
               lhsT=kq[:, si * 128:(si + 1) * 128],
               rhs=qkr[0:64, tslm],
               start=True, stop=True)
            pt = probp.tile([128, 512], BF16, tag="prob", name="pt")
            nc.scalar.activation(pt[:, off:], psc[:, off:],
                                 mybir.ActivationFunctionType.Exp,
                                 bias=s["actb"][:, si:si + 1])
            if kd >= 0:
                nc.vector.tensor_mul(pt[:, off:off + 128], pt[:, off:off + 128],
                                     tri_sb)
            pts.append(pt)
            offs.append(off)
            if filler is not None:
                filler()
            if si >= 3:
                att_mm(si - 3)
        att_mm(n_s - 3)
        att_mm(n_s - 2)
        att_mm(n_s - 1)
        att_sb = attp.tile([HEAD_DIM + 2, 512], F32R, tag="attsb", name="att_sb")
        nc.vector.tensor_copy(att_sb, patt)
        if pending:
            pending.pop(0)()
        pending.append(make_finisher(idx, tcx, att_sb))

    # ---------- interleaved pipeline across pairs ----------
    plans = {}   # idx -> [steps, pos]

    reached = {}  # idx -> set of markers already executed

    def step_one(idx):
        if idx not in plans:
            return False
        steps, pos = plans[idx]
        if pos >= len(steps):
            return False
        tag, fn = steps[pos]
        fn()
        reached.setdefault(idx, set()).add(tag)
        plans[idx][1] = pos + 1
        return True

    def drain_until(idx, marker):
        if idx not in plans:
            return
        while marker not in reached.setdefault(idx, set()):
            if not step_one(idx):
                return

    emit_tables(0)
    plans[0] = [proj_plan(0), 0]
    for _ in range(999):
        if not step_one(0):
            break
    FILLER_BUDGET = [8, 22, 40, 999]
    for idx in range(n_pairs):
        for tcx in range(NC4):
            if idx + 1 < n_pairs and tcx == 0:
                emit_tables(idx + 1)
                plans[idx + 1] = [proj_plan(idx + 1), 0]

            budget = FILLER_BUDGET[tcx]

            def filler():
                if idx + 1 in plans and plans[idx + 1][1] < budget:
                    step_one(idx + 1)

            emit_c_chunk(idx, tcx, filler)
        if idx + 1 < n_pairs:
            while step_one(idx + 1):
                pass
        if idx > 0:
            del st[idx - 1]
            del plans[idx - 1]
    while pending:
        pending.pop(0)()


_PROGRAM = None
_MM_LABELS = []


def kernel(**inputs) -> np.ndarray:
    global _PROGRAM
    in_maps = make_in_maps(inputs)
    if _PROGRAM is None:
        _PROGRAM = _build_program()
    nc = _PROGRAM
    res = run_bass_kernel_spmd(nc, in_maps, list(range(NCORES)))
    outs = [np.asarray(res.results[c]["out"]).astype(np.float32)
            for c in range(NCORES)]
    return np.concatenate(outs, axis=1)
